# revision 25
# baseline (speedup 1.0000x reference)
"""Trainium2 Bass kernel for nn_Blur2: depthwise 4x4 blur (upfirdn2d-style,
pad=(2,1), unit stride) over input [8, 128, 256, 256] f32.

Strategy: pure data parallel over the 1024 independent (n, c) planes --
128 planes per NeuronCore. Within a plane the 2D 16-tap conv runs on the
tensor engine as banded matmuls: the H-direction conv is the contraction
(banded Toeplitz fp16 weights, image rows on partitions) and the
W-direction conv is 4 shifted slices of the moving operand accumulated
into the same PSUM tile (clipped column ranges encode the zero padding,
clipped weight bands encode the H padding).

Precision: single fp16 cast of the input (tolerance is 2e-2; fp16 input
+ fp16 output quantization give ~8e-4 max rel err). Blur weights
(entries k/16) are exact in fp16; fp16 products accumulate exactly in
fp32 PSUM. The fp16 DRAM output is upcast to f32 on the host.

Engine balance: the blur kernel is separable (rank-1), so 2 of the 8
planes per tier take a W-offload path: PE does a single H-only banded
matmul into a 1-bank PSUM strip, the strip is staged to SBUF fp16
(alternating scalar/vector -- DVE may read only one PSUM operand per
op), then the 4 W taps are applied by scalar_tensor_tensor ops: two on
DVE, one on GpSimd (SBUF-only engine). The other 6 planes stay fused on
the PE (4 shifted matmuls each) so PE activity stays ~88%: the HAM
clock governor HALVES the core clock whenever activity drops for a
~3.4 us window, which slows every engine (a 4/8-plane offload attempt
regressed to 177 us that way).

DMA: 8 planes per DRAM row (4 KB fp16) and per oct the two H-tier row
blocks are host-interleaved into ONE [128, 8KB] load; both tiers' output
goes out as ONE [128, 8KB] store (row r holds out rows r and 128+r).
Fewer, larger DMAs matter twice: each DMA_DIRECT2D trigger costs ~600ns
of its engine, and a smaller instruction stream shortens the ~12 us
program-upload ramp during which the clock governor idles at half rate.
Loads ride the sync HWDGE ring, stores the scalar ring; 128-partition
DMAs split across all 16 SDMA engines (measured ~430 GB/s burst).
"""
import sys

for _p in ("/opt/trn_rl_repo", "/opt/pypackages"):
    if _p not in sys.path:
        sys.path.insert(0, _p)

import contextlib

import numpy as np


def _install_ntff_hook_shim():
    """The agent image's antenv lacks axon_hooks, which bass_utils needs
    for trace=True under axon. Provide it in sys.modules, backed by
    trn_agent_boot's ctypes NTFF shim."""
    import types

    if "antenv.axon_hooks" in sys.modules:
        return
    mod = types.ModuleType("antenv.axon_hooks")
    state = {"hook": None, "tried": False}

    def set_axon_ntff_profile_hook(hook):
        state["hook"] = hook

    def get_axon_ntff_profile_hook():
        if state["hook"] is None and not state["tried"]:
            state["tried"] = True
            try:
                from trn_agent_boot.trn_boot import _ntff_profile_via_ctypes

                state["hook"] = _ntff_profile_via_ctypes("/opt/axon/libaxon_pjrt.so")
            except Exception:
                state["hook"] = None
        return state["hook"]

    mod.set_axon_ntff_profile_hook = set_axon_ntff_profile_hook
    mod.get_axon_ntff_profile_hook = get_axon_ntff_profile_hook
    sys.modules["antenv.axon_hooks"] = mod
    try:
        import antenv

        antenv.axon_hooks = mod
    except ImportError:
        pass


_install_ntff_hook_shim()

import concourse.bacc as bacc
import concourse.tile as tile
from concourse import mybir
from concourse.bass_utils import run_bass_kernel_spmd

N_CORES = 8
H = W = 256
PLANES = 1024 // N_CORES  # 128 per core
O = 8  # planes packed per SBUF/DRAM row (4KB fp16)
NOCT = PLANES // O  # 16 oct-groups per core
QP = 4  # planes per PSUM tile in the non-separable fallback path
NOFF = 2  # planes per tier on the DVE W-offload path
NF = O - NOFF  # fused planes per tier

# M-tile layout along H per plane:
#   tier A: out rows [0, 127)   from x rows [0, 128)
#   tier B: out rows [127, 252) from x rows [125, 253)
#   remainder: out rows [252, 256) from x rows [250, 256), stacked across
#   groups of RG=16 octs (96 partitions, 4 out rows per plane-slot)
MA, MB = 127, 125
RG = 16

# per W-shift i: out cols [wl, wh), reading x cols [cl, ch)  (tap = w-2+i)
SHIFT_RANGES = {
    0: (2, 256, 0, 254),
    1: (1, 256, 0, 255),
    2: (0, 256, 0, 256),
    3: (0, 255, 1, 256),
}
SHIFT_ORDER = [2, 0, 1, 3]  # full-range shift first so start=True covers all


def _separable(wk: np.ndarray):
    """Rank-1 factorization wk = outer(uh, uw); returns (uh, uw) or None."""
    u, s, vt = np.linalg.svd(wk.astype(np.float64))
    if s[1] > 1e-6 * s[0]:
        return None
    uh = u[:, 0] * np.sqrt(s[0])
    uw = vt[0] * np.sqrt(s[0])
    if uh.sum() < 0:
        uh, uw = -uh, -uw
    if abs(uw[1]) < 1e-12 or abs(uw[2]) < 1e-12:
        return None
    return uh, uw


def _make_weights(wk: np.ndarray):
    """wk: flipped 4x4 kernel. Packed fp16 weights, one 128-col matrix per
    W-shift (cols padded with zeros past MA/MB so NumWeights==128 enables
    the PE Fast-Weight-Load path): wa/wb [128, 4*128], wr [96, 4*64]
    (block-diag 16x(6->4)). Plus H-only banded mats wa2/wb2 [128, 128]
    (taps uh*uw[2]) for the W-offload path."""
    sep = _separable(wk)
    wa2 = np.zeros((128, 128), np.float32)
    wb2 = np.zeros((128, 128), np.float32)
    if sep is not None:
        uh, uw = sep
        uh2 = (uh * uw[2]).astype(np.float32)
        for k in range(128):
            for m in range(MA):
                d = k - m + 2
                if 0 <= d <= 3:
                    wa2[k, m] = uh2[d]
            for m in range(MB):
                d = k - m
                if 0 <= d <= 3:
                    wb2[k, m] = uh2[d]
    wa = np.zeros((128, 4, 128), np.float32)
    for k in range(128):
        for m in range(MA):
            d = k - m + 2
            if 0 <= d <= 3:
                wa[k, :, m] = wk[d, :]
    wb = np.zeros((128, 4, 128), np.float32)
    for k in range(128):
        for m in range(MB):
            d = k - m
            if 0 <= d <= 3:
                wb[k, :, m] = wk[d, :]
    wr = np.zeros((RG * 6, 4, RG * 4), np.float32)
    for b in range(RG):
        for r in range(6):
            for c in range(4):
                d = r - c
                if 0 <= d <= 3:
                    wr[6 * b + r, :, 4 * b + c] = wk[d, :]
    return (
        wa.reshape(128, 4 * 128).astype(np.float16),
        wb.reshape(128, 4 * 128).astype(np.float16),
        wr.reshape(RG * 6, 4 * RG * 4).astype(np.float16),
        wa2.astype(np.float16),
        wb2.astype(np.float16),
    )


def _build_program(noct: int = NOCT, ratios=None):
    nc = bacc.Bacc("TRN2", target_bir_lowering=False, debug=False)
    f16, f32 = mybir.dt.float16, mybir.dt.float32
    offload = ratios is not None

    # xs: per oct, row r holds tier-A x row r (cols 0:O*W) interleaved with
    # tier-B x row 125+r (cols O*W:2*O*W) -> one 8KB-per-partition load.
    d_xs = nc.dram_tensor("xs", [noct, 128, 2 * O * W], f16, kind="ExternalInput").ap()
    d_xr = nc.dram_tensor("xr", [noct, 6, O * W], f16, kind="ExternalInput").ap()
    d_wa = nc.dram_tensor("wa", [128, 4 * 128], f16, kind="ExternalInput").ap()
    d_wb = nc.dram_tensor("wb", [128, 4 * 128], f16, kind="ExternalInput").ap()
    d_wr = nc.dram_tensor("wr", [RG * 6, 4 * RG * 4], f16, kind="ExternalInput").ap()
    d_wa2 = nc.dram_tensor("wa2", [128, 128], f16, kind="ExternalInput").ap()
    d_wb2 = nc.dram_tensor("wb2", [128, 128], f16, kind="ExternalInput").ap()
    # out: per oct, row r holds out rows r (tier A) and 128+r (tier B);
    # junk at (127, A) and (125..128, B). Remainder rows in d_or.
    d_out = nc.dram_tensor("out", [noct, 128, 2 * O * W], f16, kind="ExternalOutput").ap()
    d_or = nc.dram_tensor("outr", [noct, 4, O * W], f16, kind="ExternalOutput").ap()

    rem_groups = [(s, min(RG, noct - s)) for s in range(0, noct, RG)]

    with tile.TileContext(nc) as tc, contextlib.ExitStack() as ctx:
        wpool = ctx.enter_context(tc.tile_pool(name="wpool", bufs=1))
        xin = ctx.enter_context(tc.tile_pool(name="xin", bufs=5))
        xinr = ctx.enter_context(tc.tile_pool(name="xinr", bufs=2))
        psum = ctx.enter_context(tc.tile_pool(name="psum", bufs=2, space="PSUM"))
        psyp = ctx.enter_context(tc.tile_pool(name="psyp", bufs=2, space="PSUM"))
        scr = ctx.enter_context(tc.tile_pool(name="scr", bufs=3))
        outp = ctx.enter_context(tc.tile_pool(name="outp", bufs=4))
        outr = ctx.enter_context(tc.tile_pool(name="outr", bufs=2))

        # PE warmup: ~20 junk matmuls with no data dependencies, issued
        # before any real work. They run during the DMA ramp and lift the
        # HAM clock gate to 2.4 GHz before the real stream starts.
        warm = wpool.tile([128, 2 * W], f16, tag="warm")
        nc.vector.memset(warm[:], 0.0)
        psW = psum.tile([128, 2 * W], f32, tag="psA")
        for _ in range(24):
            nc.tensor.matmul(
                psW[:, :], warm[:, :128], warm[:, :],
                start=True, stop=True, skip_group_check=True,
            )

        t_wa2 = wpool.tile([128, 128], f16, tag="wa2")
        nc.scalar.dma_start(out=t_wa2[:], in_=d_wa2)
        t_wb2 = wpool.tile([128, 128], f16, tag="wb2")
        nc.scalar.dma_start(out=t_wb2[:], in_=d_wb2)
        t_wa = wpool.tile([128, 4 * 128], f16, tag="wa")
        nc.scalar.dma_start(out=t_wa[:], in_=d_wa)
        t_wb = wpool.tile([128, 4 * 128], f16, tag="wb")
        nc.scalar.dma_start(out=t_wb[:], in_=d_wb)
        t_wr = wpool.tile([RG * 6, 4 * RG * 4], f16, tag="wr")
        nc.scalar.dma_start(out=t_wr[:], in_=d_wr)

        def conv_mms(ps, wt, xt, xrows, qbase, npl=QP):
            """4 shifts x npl planes accumulating matmuls into the psum tile
            ps [128, npl*W] (per-bank first mm gets start=True). Plane p of
            the merged input tile xt sits at cols [p*W, (p+1)*W)."""
            last = (SHIFT_ORDER[-1], npl - 1)
            for i in SHIFT_ORDER:
                wl, wh, cl, ch = SHIFT_RANGES[i]
                lhsT = wt[:xrows, i * 128 : i * 128 + 128]
                if i == 2 and npl % 2 == 0:
                    # full-range shift: adjacent planes are contiguous in both
                    # psum and x, so run 512-col matmuls (one whole PSUM bank
                    # each; this shift goes first so start=True arms the bank)
                    for qq in range(0, npl, 2):
                        nc.tensor.matmul(
                            ps[:128, qq * W : (qq + 2) * W],
                            lhsT,
                            xt[:xrows, (qbase + qq) * W : (qbase + qq + 2) * W],
                            start=True,
                            stop=False,
                            skip_group_check=True,
                        )
                    continue
                for q in range(npl):
                    nc.tensor.matmul(
                        ps[:128, q * W + wl : q * W + wh],
                        lhsT,
                        xt[:xrows, (qbase + q) * W + cl : (qbase + q) * W + ch],
                        start=(i == SHIFT_ORDER[0] and q % 2 == 0),
                        stop=((i, q) == last),
                        skip_group_check=True,
                    )

        # Static y-staging strips for the offload path: borders (W zero-pad)
        # are memset ONCE here and never rewritten -- the per-plane stage
        # copy only fills cols [2, 258), so tap reads strip[:, i:i+256] see
        # zeros at the pad positions. 4 strips rotated manually give the
        # same pipelining as a pool without per-use border memsets.
        strips = []
        if offload:
            for si in range(4):
                st = wpool.tile([128, 260], f16, tag=f"st{si}")
                nc.vector.memset(st[:, 0:2], 0.0)
                nc.vector.memset(st[:, 258:260], 0.0)
                strips.append(st)
        pc = [0]

        def offload_plane(o, wt2, xt, p):
            """Separable path for plane p of the merged tile: PE computes the
            H-only conv y (scaled by uw2) into a half-bank PSUM tile; y is
            staged to two SBUF fp16 strips offset by one column (so all tap
            reads are 4B-aligned for the DVE 2x uop mode; DVE also reads at
            most one PSUM operand per op), then the symmetric W taps are two
            pairwise adds (GpSimd + DVE) and one DVE scalar_tensor_tensor
            writing the fp16 output tile directly."""
            r01, r32, r12 = ratios
            mult, add = mybir.AluOpType.mult, mybir.AluOpType.add
            strip = strips[pc[0] % 4]
            stage_scalar = pc[0] % 2 == 0
            pc[0] += 1
            psy = psyp.tile([128, 256], f32, tag="psy")
            nc.tensor.matmul(
                psy[:128, 0:256], wt2[:128, :], xt[:128, p * W : (p + 1) * W],
                start=True, stop=True, skip_group_check=True,
            )
            if stage_scalar:
                nc.scalar.copy(strip[:, 2:258], psy[:, 0:256])
            else:
                nc.vector.tensor_copy(strip[:, 2:258], psy[:, 0:256])
            sc = scr.tile([128, 512], f16, tag="sc")
            nc.vector.scalar_tensor_tensor(
                out=sc[:, 0:256], in0=strip[:, 0:256], scalar=r01,
                in1=strip[:, 1:257], op0=mult, op1=add,
            )
            nc.vector.scalar_tensor_tensor(
                out=sc[:, 256:512], in0=strip[:, 3:259], scalar=r32,
                in1=strip[:, 2:258], op0=mult, op1=add,
            )
            nc.vector.scalar_tensor_tensor(
                out=o[:, p * W : (p + 1) * W], in0=sc[:, 0:256], scalar=r12,
                in1=sc[:, 256:512], op0=mult, op1=add,
            )

        ri = 0
        for g in range(noct):
            tab = xin.tile([128, 2 * O * W], f16, tag="tab")
            if g == 0:
                # split the very first load so the PE stream starts sooner
                nc.sync.dma_start(out=tab[0:64, :], in_=d_xs[g, 0:64, :])
                nc.sync.dma_start(out=tab[64:128, :], in_=d_xs[g, 64:128, :])
            else:
                nc.sync.dma_start(out=tab[:], in_=d_xs[g])

            oab = outp.tile([128, 2 * O * W], f16, tag="oab")
            for tier in range(2):
                wt, wt2 = (t_wa, t_wa2) if tier == 0 else (t_wb, t_wb2)
                base = tier * O
                if offload:
                    for p in range(NF, O):
                        offload_plane(oab, wt2, tab, base + p)
                    ps = psum.tile([128, NF * W], f32, tag="psA")
                    conv_mms(ps, wt, tab, 128, base, NF)
                    if g == noct - 1:
                        # last oct is the kernel tail (runs at half clock once
                        # PE goes quiet): split evac across engines and store
                        # each tier half as soon as it is ready
                        hw = NF * W // 2
                        lo = base * W
                        nc.scalar.copy(oab[:, lo : lo + hw], ps[:, :hw])
                        nc.vector.tensor_copy(oab[:, lo + hw : lo + 2 * hw], ps[:, hw:])
                        nc.scalar.dma_start(
                            out=d_out[g, :, base * W : (base + O) * W],
                            in_=oab[:, base * W : (base + O) * W],
                        )
                    else:
                        nc.scalar.copy(oab[:, base * W : (base + NF) * W], ps[:, :])
                else:
                    for h in range(O // QP):
                        ps = psum.tile([128, QP * W], f32, tag="psA")
                        conv_mms(ps, wt, tab, 128, base + h * QP)
                        hw = QP * W // 2
                        lo = (base + h * QP) * W
                        if (g + h + tier) % 2 == 0:
                            nc.scalar.copy(oab[:, lo : lo + hw], ps[:, :hw])
                            nc.vector.tensor_copy(oab[:, lo + hw : lo + 2 * hw], ps[:, hw:])
                        else:
                            nc.vector.tensor_copy(oab[:, lo : lo + hw], ps[:, :hw])
                            nc.scalar.copy(oab[:, lo + hw : lo + 2 * hw], ps[:, hw:])
            if g != noct - 1 or not offload:
                nc.scalar.dma_start(out=d_out[g], in_=oab[:])

            # stacked remainder: input rows come straight from DRAM, so
            # emit early (oct 2, 4, ...) to keep them off the kernel tail
            if ri < len(rem_groups) and g == min(2 * (ri + 1), noct - 1):
                s, gsz = rem_groups[ri]
                ri += 1
                tr = xinr.tile([RG * 6, O * W], f16, tag="tr")
                nc.sync.dma_start(out=tr[: 6 * gsz, :], in_=d_xr[s : s + gsz])
                orr = outr.tile([RG * 4, O * W], f16, tag="orr")
                for h in range(O // QP):
                    psR = psum.tile([RG * 4, QP * W], f32, tag="psA")
                    last = (SHIFT_ORDER[-1], QP - 1)
                    for i in SHIFT_ORDER:
                        wl, wh, cl, ch = SHIFT_RANGES[i]
                        lhsT = t_wr[: 6 * gsz, i * RG * 4 : i * RG * 4 + 4 * gsz]
                        for q in range(QP):
                            nc.tensor.matmul(
                                psR[: 4 * gsz, q * W + wl : q * W + wh],
                                lhsT,
                                tr[: 6 * gsz, (h * QP + q) * W + cl : (h * QP + q) * W + ch],
                                start=(i == SHIFT_ORDER[0] and q % 2 == 0),
                                stop=((i, q) == last),
                                skip_group_check=True,
                            )
                    if (g + h) % 2 == 0:
                        nc.scalar.copy(
                            orr[: 4 * gsz, h * QP * W : (h + 1) * QP * W],
                            psR[: 4 * gsz, :],
                        )
                    else:
                        nc.vector.tensor_copy(
                            orr[: 4 * gsz, h * QP * W : (h + 1) * QP * W],
                            psR[: 4 * gsz, :],
                        )
                nc.scalar.dma_start(out=d_or[s : s + gsz], in_=orr[: 4 * gsz])

    nc.compile()
    return nc


_CACHE = {}


def _get_program(noct: int = NOCT, ratios=None):
    key = (noct, ratios)
    if key not in _CACHE:
        _CACHE[key] = _build_program(noct, ratios)
    return _CACHE[key]


def _run(x: np.ndarray, wk: np.ndarray, trace: bool = False):
    """x: [P, 256, 256] f32 full stack of planes (P divisible by 8*O),
    wk: flipped 4x4 kernel. Returns ([P, 256, 256] f32, exec_time_ns|None)."""
    P = x.shape[0]
    oper = P // (N_CORES * O)
    hi = x.astype(np.float16)
    # oct-pack: [P/O, O, H, W] -> [P/O, H, O, W] -> [P/O, H, O*W]
    xso = (
        hi.reshape(P // O, O, H, W)
        .transpose(0, 2, 1, 3)
        .reshape(P // O, H, O * W)
    )
    # interleave tier-A rows 0:128 with tier-B rows 125:253 -> 8KB DMA rows
    xs3 = np.stack([xso[:, 0:128], xso[:, 125:253]], axis=2).reshape(
        P // O, 128, 2 * O * W
    )
    xrem = np.ascontiguousarray(xso[:, 250:256])  # [P/O, 6, O*W]

    wa, wb, wr, wa2, wb2 = _make_weights(wk)
    sep = _separable(wk)
    ratios = None
    if sep is not None:
        uh, uw = sep
        ratios = (
            float(uw[0] / uw[1]),
            float(uw[3] / uw[2]),
            float(uw[1] / uw[2]),
        )
    nc = _get_program(oper, ratios)

    in_maps = [
        {
            "xs": np.ascontiguousarray(xs3[c * oper : (c + 1) * oper]),
            "xr": xrem[c * oper : (c + 1) * oper],
            "wa": wa,
            "wb": wb,
            "wr": wr,
            "wa2": wa2,
            "wb2": wb2,
        }
        for c in range(N_CORES)
    ]
    res = run_bass_kernel_spmd(nc, in_maps, list(range(N_CORES)), trace=trace)
    outq = np.concatenate([r["out"] for r in res.results], axis=0)
    outq = outq.reshape(P // O, 128, 2, O * W)
    outrem = np.concatenate([r["outr"] for r in res.results], axis=0)  # [P/O,4,O*W]
    full = np.concatenate(
        [outq[:, 0:127, 0], outq[:, 0:125, 1], outrem], axis=1
    )  # [P/O, 256, O*W]
    out = (
        full.reshape(P // O, H, O, W)
        .transpose(0, 2, 1, 3)
        .reshape(P, H, W)
        .astype(np.float32)
    )
    return np.ascontiguousarray(out), res.exec_time_ns


def kernel(input: np.ndarray, kernel: np.ndarray) -> np.ndarray:
    x = np.asarray(input, dtype=np.float32)
    k = np.asarray(kernel, dtype=np.float32)
    n, c, h, w = x.shape
    wk = np.flip(k, (0, 1)).copy()  # correlation weights
    out, _ = _run(x.reshape(n * c, h, w), wk, trace=False)
    return out.reshape(n, c, h, w)


# revision 26
# speedup vs baseline: 1.0286x; 1.0286x over previous
"""Trainium2 Bass kernel for nn_Blur2: depthwise 4x4 blur (upfirdn2d-style,
pad=(2,1), unit stride) over input [8, 128, 256, 256] f32.

Strategy: pure data parallel over the 1024 independent (n, c) planes --
128 planes per NeuronCore. Within a plane the 2D 16-tap conv runs on the
tensor engine as banded matmuls: the H-direction conv is the contraction
(banded Toeplitz fp16 weights, image rows on partitions) and the
W-direction conv is 4 shifted slices of the moving operand accumulated
into the same PSUM tile (clipped column ranges encode the zero padding,
clipped weight bands encode the H padding).

Precision: single fp16 cast of the input (tolerance is 2e-2; fp16 input
+ fp16 output quantization give ~8e-4 max rel err). Blur weights
(entries k/16) are exact in fp16; fp16 products accumulate exactly in
fp32 PSUM. The fp16 DRAM output is upcast to f32 on the host.

Engine balance: the blur kernel is separable (rank-1), so 2 of the 8
planes per tier take a W-offload path: PE does a single H-only banded
matmul into a 1-bank PSUM strip, the strip is staged to SBUF fp16
(alternating scalar/vector -- DVE may read only one PSUM operand per
op), then the 4 W taps are applied by scalar_tensor_tensor ops: two on
DVE, one on GpSimd (SBUF-only engine). The other 6 planes stay fused on
the PE (4 shifted matmuls each) so PE activity stays ~88%: the HAM
clock governor HALVES the core clock whenever activity drops for a
~3.4 us window, which slows every engine (a 4/8-plane offload attempt
regressed to 177 us that way).

DMA: 8 planes per DRAM row (4 KB fp16) and per oct the two H-tier row
blocks are host-interleaved into ONE [128, 8KB] load; both tiers' output
goes out as ONE [128, 8KB] store (row r holds out rows r and 128+r).
Fewer, larger DMAs matter twice: each DMA_DIRECT2D trigger costs ~600ns
of its engine, and a smaller instruction stream shortens the ~12 us
program-upload ramp during which the clock governor idles at half rate.
Loads ride the sync HWDGE ring, stores the scalar ring; 128-partition
DMAs split across all 16 SDMA engines (measured ~430 GB/s burst).
"""
import sys

for _p in ("/opt/trn_rl_repo", "/opt/pypackages"):
    if _p not in sys.path:
        sys.path.insert(0, _p)

import contextlib

import numpy as np


def _install_ntff_hook_shim():
    """The agent image's antenv lacks axon_hooks, which bass_utils needs
    for trace=True under axon. Provide it in sys.modules, backed by
    trn_agent_boot's ctypes NTFF shim."""
    import types

    if "antenv.axon_hooks" in sys.modules:
        return
    mod = types.ModuleType("antenv.axon_hooks")
    state = {"hook": None, "tried": False}

    def set_axon_ntff_profile_hook(hook):
        state["hook"] = hook

    def get_axon_ntff_profile_hook():
        if state["hook"] is None and not state["tried"]:
            state["tried"] = True
            try:
                from trn_agent_boot.trn_boot import _ntff_profile_via_ctypes

                state["hook"] = _ntff_profile_via_ctypes("/opt/axon/libaxon_pjrt.so")
            except Exception:
                state["hook"] = None
        return state["hook"]

    mod.set_axon_ntff_profile_hook = set_axon_ntff_profile_hook
    mod.get_axon_ntff_profile_hook = get_axon_ntff_profile_hook
    sys.modules["antenv.axon_hooks"] = mod
    try:
        import antenv

        antenv.axon_hooks = mod
    except ImportError:
        pass


_install_ntff_hook_shim()

import concourse.bacc as bacc
import concourse.tile as tile
from concourse import mybir
from concourse.bass_utils import run_bass_kernel_spmd

N_CORES = 8
H = W = 256
PLANES = 1024 // N_CORES  # 128 per core
O = 8  # planes packed per SBUF/DRAM row (4KB fp16)
NOCT = PLANES // O  # 16 oct-groups per core
QP = 4  # planes per PSUM tile in the non-separable fallback path
NOFF = 2  # planes per tier on the DVE W-offload path
NF = O - NOFF  # fused planes per tier

# M-tile layout along H per plane:
#   tier A: out rows [0, 127)   from x rows [0, 128)
#   tier B: out rows [127, 252) from x rows [125, 253)
#   remainder: out rows [252, 256) from x rows [250, 256), stacked across
#   groups of RG=16 octs (96 partitions, 4 out rows per plane-slot)
MA, MB = 127, 125
RG = 16

# per W-shift i: out cols [wl, wh), reading x cols [cl, ch)  (tap = w-2+i)
SHIFT_RANGES = {
    0: (2, 256, 0, 254),
    1: (1, 256, 0, 255),
    2: (0, 256, 0, 256),
    3: (0, 255, 1, 256),
}
SHIFT_ORDER = [2, 0, 1, 3]  # full-range shift first so start=True covers all


def _separable(wk: np.ndarray):
    """Rank-1 factorization wk = outer(uh, uw); returns (uh, uw) or None."""
    u, s, vt = np.linalg.svd(wk.astype(np.float64))
    if s[1] > 1e-6 * s[0]:
        return None
    uh = u[:, 0] * np.sqrt(s[0])
    uw = vt[0] * np.sqrt(s[0])
    if uh.sum() < 0:
        uh, uw = -uh, -uw
    if abs(uw[1]) < 1e-12 or abs(uw[2]) < 1e-12:
        return None
    return uh, uw


def _make_weights(wk: np.ndarray):
    """wk: flipped 4x4 kernel. Packed fp16 weights, one 128-col matrix per
    W-shift (cols padded with zeros past MA/MB so NumWeights==128 enables
    the PE Fast-Weight-Load path): wa/wb [128, 4*128], wr [96, 4*64]
    (block-diag 16x(6->4)). Plus H-only banded mats wa2/wb2 [128, 128]
    (taps uh*uw[2]) for the W-offload path."""
    sep = _separable(wk)
    wa2 = np.zeros((128, 128), np.float32)
    wb2 = np.zeros((128, 128), np.float32)
    if sep is not None:
        uh, uw = sep
        uh2 = (uh * uw[2]).astype(np.float32)
        for k in range(128):
            for m in range(MA):
                d = k - m + 2
                if 0 <= d <= 3:
                    wa2[k, m] = uh2[d]
            for m in range(MB):
                d = k - m
                if 0 <= d <= 3:
                    wb2[k, m] = uh2[d]
    wa = np.zeros((128, 4, 128), np.float32)
    for k in range(128):
        for m in range(MA):
            d = k - m + 2
            if 0 <= d <= 3:
                wa[k, :, m] = wk[d, :]
    wb = np.zeros((128, 4, 128), np.float32)
    for k in range(128):
        for m in range(MB):
            d = k - m
            if 0 <= d <= 3:
                wb[k, :, m] = wk[d, :]
    wr = np.zeros((RG * 6, 4, RG * 4), np.float32)
    for b in range(RG):
        for r in range(6):
            for c in range(4):
                d = r - c
                if 0 <= d <= 3:
                    wr[6 * b + r, :, 4 * b + c] = wk[d, :]
    return (
        wa.reshape(128, 4 * 128).astype(np.float16),
        wb.reshape(128, 4 * 128).astype(np.float16),
        wr.reshape(RG * 6, 4 * RG * 4).astype(np.float16),
        wa2.astype(np.float16),
        wb2.astype(np.float16),
    )


def _build_program(noct: int = NOCT, ratios=None):
    nc = bacc.Bacc("TRN2", target_bir_lowering=False, debug=False)
    f16, f32 = mybir.dt.float16, mybir.dt.float32
    offload = ratios is not None

    # xs: per oct, row r holds tier-A x row r (cols 0:O*W) interleaved with
    # tier-B x row 125+r (cols O*W:2*O*W) -> one 8KB-per-partition load.
    d_xs = nc.dram_tensor("xs", [noct, 128, 2 * O * W], f16, kind="ExternalInput").ap()
    d_xr = nc.dram_tensor("xr", [noct, 6, O * W], f16, kind="ExternalInput").ap()
    d_wa = nc.dram_tensor("wa", [128, 4 * 128], f16, kind="ExternalInput").ap()
    d_wb = nc.dram_tensor("wb", [128, 4 * 128], f16, kind="ExternalInput").ap()
    d_wr = nc.dram_tensor("wr", [RG * 6, 4 * RG * 4], f16, kind="ExternalInput").ap()
    d_wa2 = nc.dram_tensor("wa2", [128, 128], f16, kind="ExternalInput").ap()
    d_wb2 = nc.dram_tensor("wb2", [128, 128], f16, kind="ExternalInput").ap()
    # out: per oct, row r holds out rows r (tier A) and 128+r (tier B);
    # junk at (127, A) and (125..128, B). Remainder rows in d_or.
    d_out = nc.dram_tensor("out", [noct, 128, 2 * O * W], f16, kind="ExternalOutput").ap()
    d_or = nc.dram_tensor("outr", [noct, 4, O * W], f16, kind="ExternalOutput").ap()

    rem_groups = [(s, min(RG, noct - s)) for s in range(0, noct, RG)]

    with tile.TileContext(nc) as tc, contextlib.ExitStack() as ctx:
        wpool = ctx.enter_context(tc.tile_pool(name="wpool", bufs=1))
        xin = ctx.enter_context(tc.tile_pool(name="xin", bufs=5))
        xinr = ctx.enter_context(tc.tile_pool(name="xinr", bufs=2))
        psum = ctx.enter_context(tc.tile_pool(name="psum", bufs=2, space="PSUM"))
        psyp = ctx.enter_context(tc.tile_pool(name="psyp", bufs=2, space="PSUM"))
        scr = ctx.enter_context(tc.tile_pool(name="scr", bufs=3))
        outp = ctx.enter_context(tc.tile_pool(name="outp", bufs=4))
        outr = ctx.enter_context(tc.tile_pool(name="outr", bufs=2))

        # PE warmup: ~20 junk matmuls with no data dependencies, issued
        # before any real work. They run during the DMA ramp and lift the
        # HAM clock gate to 2.4 GHz before the real stream starts.
        warm = wpool.tile([128, 2 * W], f16, tag="warm")
        nc.vector.memset(warm[:], 0.0)
        psW = psum.tile([128, 2 * W], f32, tag="psA")
        for _ in range(24):
            nc.tensor.matmul(
                psW[:, :], warm[:, :128], warm[:, :],
                start=True, stop=True, skip_group_check=True,
            )

        t_wa2 = wpool.tile([128, 128], f16, tag="wa2")
        nc.scalar.dma_start(out=t_wa2[:], in_=d_wa2)
        t_wb2 = wpool.tile([128, 128], f16, tag="wb2")
        nc.scalar.dma_start(out=t_wb2[:], in_=d_wb2)
        t_wa = wpool.tile([128, 4 * 128], f16, tag="wa")
        nc.scalar.dma_start(out=t_wa[:], in_=d_wa)
        t_wb = wpool.tile([128, 4 * 128], f16, tag="wb")
        nc.scalar.dma_start(out=t_wb[:], in_=d_wb)
        t_wr = wpool.tile([RG * 6, 4 * RG * 4], f16, tag="wr")
        nc.scalar.dma_start(out=t_wr[:], in_=d_wr)

        def conv_mms(ps, wt, xt, xrows, qbase, npl=QP):
            """4 shifts x npl planes accumulating matmuls into the psum tile
            ps [128, npl*W] (per-bank first mm gets start=True). Plane p of
            the merged input tile xt sits at cols [p*W, (p+1)*W)."""
            last = (SHIFT_ORDER[-1], npl - 1)
            for i in SHIFT_ORDER:
                wl, wh, cl, ch = SHIFT_RANGES[i]
                lhsT = wt[:xrows, i * 128 : i * 128 + 128]
                if i == 2 and npl % 2 == 0:
                    # full-range shift: adjacent planes are contiguous in both
                    # psum and x, so run 512-col matmuls (one whole PSUM bank
                    # each; this shift goes first so start=True arms the bank)
                    for qq in range(0, npl, 2):
                        nc.tensor.matmul(
                            ps[:128, qq * W : (qq + 2) * W],
                            lhsT,
                            xt[:xrows, (qbase + qq) * W : (qbase + qq + 2) * W],
                            start=True,
                            stop=False,
                            skip_group_check=True,
                        )
                    continue
                for q in range(npl):
                    nc.tensor.matmul(
                        ps[:128, q * W + wl : q * W + wh],
                        lhsT,
                        xt[:xrows, (qbase + q) * W + cl : (qbase + q) * W + ch],
                        start=(i == SHIFT_ORDER[0] and q % 2 == 0),
                        stop=((i, q) == last),
                        skip_group_check=True,
                    )

        # Static y-staging strips for the offload path: borders (W zero-pad)
        # are memset ONCE here and never rewritten -- the per-plane stage
        # copy only fills cols [2, 258), so tap reads strip[:, i:i+256] see
        # zeros at the pad positions. 4 strips rotated manually give the
        # same pipelining as a pool without per-use border memsets.
        strips = []
        if offload:
            for si in range(4):
                st = wpool.tile([128, 260], f16, tag=f"st{si}")
                nc.vector.memset(st[:, 0:2], 0.0)
                nc.vector.memset(st[:, 258:260], 0.0)
                strips.append(st)
        pc = [0]

        def offload_plane(o, wt2, xt, p):
            """Separable path for plane p of the merged tile: PE computes the
            H-only conv y (scaled by uw2) into a half-bank PSUM tile; y is
            staged to two SBUF fp16 strips offset by one column (so all tap
            reads are 4B-aligned for the DVE 2x uop mode; DVE also reads at
            most one PSUM operand per op), then the symmetric W taps are two
            pairwise adds (GpSimd + DVE) and one DVE scalar_tensor_tensor
            writing the fp16 output tile directly."""
            r01, r32, r12 = ratios
            mult, add = mybir.AluOpType.mult, mybir.AluOpType.add
            strip = strips[pc[0] % 4]
            stage_scalar = pc[0] % 2 == 0
            pc[0] += 1
            psy = psyp.tile([128, 256], f32, tag="psy")
            nc.tensor.matmul(
                psy[:128, 0:256], wt2[:128, :], xt[:128, p * W : (p + 1) * W],
                start=True, stop=True, skip_group_check=True,
            )
            if stage_scalar:
                nc.scalar.copy(strip[:, 2:258], psy[:, 0:256])
            else:
                nc.vector.tensor_copy(strip[:, 2:258], psy[:, 0:256])
            sc = scr.tile([128, 512], f16, tag="sc")
            nc.vector.scalar_tensor_tensor(
                out=sc[:, 0:256], in0=strip[:, 0:256], scalar=r01,
                in1=strip[:, 1:257], op0=mult, op1=add,
            )
            nc.vector.scalar_tensor_tensor(
                out=sc[:, 256:512], in0=strip[:, 3:259], scalar=r32,
                in1=strip[:, 2:258], op0=mult, op1=add,
            )
            nc.vector.scalar_tensor_tensor(
                out=o[:, p * W : (p + 1) * W], in0=sc[:, 0:256], scalar=r12,
                in1=sc[:, 256:512], op0=mult, op1=add,
            )

        ri = 0
        for g in range(noct):
            tab = xin.tile([128, 2 * O * W], f16, tag="tab")
            if g == 0:
                # split the very first load so the PE stream starts sooner
                nc.sync.dma_start(out=tab[0:64, :], in_=d_xs[g, 0:64, :])
                nc.sync.dma_start(out=tab[64:128, :], in_=d_xs[g, 64:128, :])
            else:
                nc.sync.dma_start(out=tab[:], in_=d_xs[g])

            oab = outp.tile([128, 2 * O * W], f16, tag="oab")
            for tier in range(2):
                wt, wt2 = (t_wa, t_wa2) if tier == 0 else (t_wb, t_wb2)
                base = tier * O
                if offload:
                    for p in range(NF, O):
                        offload_plane(oab, wt2, tab, base + p)
                    ps = psum.tile([128, NF * W], f32, tag="psA")
                    conv_mms(ps, wt, tab, 128, base, NF)
                    nc.scalar.copy(oab[:, base * W : (base + NF) * W], ps[:, :])
                else:
                    for h in range(O // QP):
                        ps = psum.tile([128, QP * W], f32, tag="psA")
                        conv_mms(ps, wt, tab, 128, base + h * QP)
                        hw = QP * W // 2
                        lo = (base + h * QP) * W
                        if (g + h + tier) % 2 == 0:
                            nc.scalar.copy(oab[:, lo : lo + hw], ps[:, :hw])
                            nc.vector.tensor_copy(oab[:, lo + hw : lo + 2 * hw], ps[:, hw:])
                        else:
                            nc.vector.tensor_copy(oab[:, lo : lo + hw], ps[:, :hw])
                            nc.scalar.copy(oab[:, lo + hw : lo + 2 * hw], ps[:, hw:])
            nc.scalar.dma_start(out=d_out[g], in_=oab[:])

            # stacked remainder: input rows come straight from DRAM, so
            # emit early (oct 2, 4, ...) to keep them off the kernel tail
            if ri < len(rem_groups) and g == min(2 * (ri + 1), noct - 1):
                s, gsz = rem_groups[ri]
                ri += 1
                tr = xinr.tile([RG * 6, O * W], f16, tag="tr")
                nc.sync.dma_start(out=tr[: 6 * gsz, :], in_=d_xr[s : s + gsz])
                orr = outr.tile([RG * 4, O * W], f16, tag="orr")
                for h in range(O // QP):
                    psR = psum.tile([RG * 4, QP * W], f32, tag="psA")
                    last = (SHIFT_ORDER[-1], QP - 1)
                    for i in SHIFT_ORDER:
                        wl, wh, cl, ch = SHIFT_RANGES[i]
                        lhsT = t_wr[: 6 * gsz, i * RG * 4 : i * RG * 4 + 4 * gsz]
                        for q in range(QP):
                            nc.tensor.matmul(
                                psR[: 4 * gsz, q * W + wl : q * W + wh],
                                lhsT,
                                tr[: 6 * gsz, (h * QP + q) * W + cl : (h * QP + q) * W + ch],
                                start=(i == SHIFT_ORDER[0] and q % 2 == 0),
                                stop=((i, q) == last),
                                skip_group_check=True,
                            )
                    if (g + h) % 2 == 0:
                        nc.scalar.copy(
                            orr[: 4 * gsz, h * QP * W : (h + 1) * QP * W],
                            psR[: 4 * gsz, :],
                        )
                    else:
                        nc.vector.tensor_copy(
                            orr[: 4 * gsz, h * QP * W : (h + 1) * QP * W],
                            psR[: 4 * gsz, :],
                        )
                nc.scalar.dma_start(out=d_or[s : s + gsz], in_=orr[: 4 * gsz])

    nc.compile()
    return nc


_CACHE = {}


def _get_program(noct: int = NOCT, ratios=None):
    key = (noct, ratios)
    if key not in _CACHE:
        _CACHE[key] = _build_program(noct, ratios)
    return _CACHE[key]


def _run(x: np.ndarray, wk: np.ndarray, trace: bool = False):
    """x: [P, 256, 256] f32 full stack of planes (P divisible by 8*O),
    wk: flipped 4x4 kernel. Returns ([P, 256, 256] f32, exec_time_ns|None)."""
    P = x.shape[0]
    oper = P // (N_CORES * O)
    hi = x.astype(np.float16)
    # oct-pack: [P/O, O, H, W] -> [P/O, H, O, W] -> [P/O, H, O*W]
    xso = (
        hi.reshape(P // O, O, H, W)
        .transpose(0, 2, 1, 3)
        .reshape(P // O, H, O * W)
    )
    # interleave tier-A rows 0:128 with tier-B rows 125:253 -> 8KB DMA rows
    xs3 = np.stack([xso[:, 0:128], xso[:, 125:253]], axis=2).reshape(
        P // O, 128, 2 * O * W
    )
    xrem = np.ascontiguousarray(xso[:, 250:256])  # [P/O, 6, O*W]

    wa, wb, wr, wa2, wb2 = _make_weights(wk)
    sep = _separable(wk)
    ratios = None
    if sep is not None:
        uh, uw = sep
        ratios = (
            float(uw[0] / uw[1]),
            float(uw[3] / uw[2]),
            float(uw[1] / uw[2]),
        )
    nc = _get_program(oper, ratios)

    in_maps = [
        {
            "xs": np.ascontiguousarray(xs3[c * oper : (c + 1) * oper]),
            "xr": xrem[c * oper : (c + 1) * oper],
            "wa": wa,
            "wb": wb,
            "wr": wr,
            "wa2": wa2,
            "wb2": wb2,
        }
        for c in range(N_CORES)
    ]
    res = run_bass_kernel_spmd(nc, in_maps, list(range(N_CORES)), trace=trace)
    outq = np.concatenate([r["out"] for r in res.results], axis=0)
    outq = outq.reshape(P // O, 128, 2, O * W)
    outrem = np.concatenate([r["outr"] for r in res.results], axis=0)  # [P/O,4,O*W]
    full = np.concatenate(
        [outq[:, 0:127, 0], outq[:, 0:125, 1], outrem], axis=1
    )  # [P/O, 256, O*W]
    out = (
        full.reshape(P // O, H, O, W)
        .transpose(0, 2, 1, 3)
        .reshape(P, H, W)
        .astype(np.float32)
    )
    return np.ascontiguousarray(out), res.exec_time_ns


def kernel(input: np.ndarray, kernel: np.ndarray) -> np.ndarray:
    x = np.asarray(input, dtype=np.float32)
    k = np.asarray(kernel, dtype=np.float32)
    n, c, h, w = x.shape
    wk = np.flip(k, (0, 1)).copy()  # correlation weights
    out, _ = _run(x.reshape(n * c, h, w), wk, trace=False)
    return out.reshape(n, c, h, w)
